# revision 5
# baseline (speedup 1.0000x reference)
"""BarlowTwinsLoss on 8 Trainium2 NeuronCores.

Math: with xs = standardize(X1), ys = standardize(X2) (per-feature batch
stats, ddof=1), C = cos-sim matrix of rows: C[i,j] = u_i . v_j where
u_i = xs_i/|xs_i|, v_j = ys_j/|ys_j|.  The loss only needs
  inv_term = (N - sum_i C_ii)/N
  red_term = LAM/N * (sum_ij C_ij^2 - sum_i C_ii^2)
and sum_ij C_ij^2 = <U^T U, V^T V>_F, which collapses the O(N^2 F) problem
to O(N F^2): two [F,F] Gram matrices.

Distribution: rows sharded 8 ways, shipped to the cores as bf16 (halves
the per-core H2D staging, which is what staggers the 8 core starts and
therefore what the first collective's rendezvous waits on).  Per core:
partial column moments -> AllGather + on-core fold (global mu/sd) ->
standardize local rows -> local Gram partials A_c, B_c [64,64] + diag
partials -> ReduceScatter so core k holds 8 feature-rows of the global
A and B -> per-core partial scalar loss.  Host sums the 8 partial losses
(the "all-reduce the scalar partial losses" step of the sharding hint).
"""

import numpy as np

N_CORES = 8
N_TOTAL = 16384
F = 64
ROWS = N_TOTAL // N_CORES  # 2048 rows per core
J = 16                     # free-dim row-chunks per partition: 128 * 16 = 2048
LAM = 0.2

_BUILT = {}


def _build_bass():
    import concourse.bacc as bacc
    import concourse.mybir as mybir
    import concourse.tile as tile

    fp32 = mybir.dt.float32
    bf16 = mybir.dt.bfloat16
    mult = mybir.AluOpType.mult
    add = mybir.AluOpType.add
    subtract = mybir.AluOpType.subtract
    bypass = mybir.AluOpType.bypass
    AX = mybir.AxisListType.X

    nc = bacc.Bacc(
        "TRN2", target_bir_lowering=False, debug=False, num_devices=N_CORES
    )

    x1_d = nc.dram_tensor("x1", [ROWS, F], bf16, kind="ExternalInput")
    x2_d = nc.dram_tensor("x2", [ROWS, F], bf16, kind="ExternalInput")
    out_d = nc.dram_tensor("out", [1, 1], fp32, kind="ExternalOutput")

    rg = [list(range(N_CORES))]
    Nf = float(N_TOTAL)

    with tile.TileContext(nc) as tc:
        with (
            tc.tile_pool(name="sb", bufs=1) as sb,
            tc.tile_pool(name="ps", bufs=1, space="PSUM") as ps,
            tc.tile_pool(name="dram", bufs=1, space="DRAM") as dram,
        ):
            # ---- constants ----
            ones_bf = sb.tile([128, 1], bf16)
            ones_fr = sb.tile([1, 128], fp32)   # row of ones (K=1 bcast matmuls)
            ones_fc = sb.tile([128, 1], fp32)   # column of ones (partition folds)
            nc.vector.memset(ones_bf[:], 1.0)
            nc.vector.memset(ones_fr[:], 1.0)
            nc.vector.memset(ones_fc[:], 1.0)

            # ---- load inputs: [2048,64] -> [128 partitions, 16 chunks, 64] ----
            # partition p holds rows p*16 .. p*16+15 (2KB contiguous per partition)
            x1b = sb.tile([128, J, F], bf16)
            x2b = sb.tile([128, J, F], bf16)
            nc.sync.dma_start(x1b[:], x1_d.ap().rearrange("(p j) f -> p j f", p=128))
            nc.sync.dma_start(x2b[:], x2_d.ap().rearrange("(p j) f -> p j f", p=128))

            # ---- squares on ACT ----
            sq1 = sb.tile([128, J, F], bf16)
            sq2 = sb.tile([128, J, F], bf16)
            nc.scalar.square(sq1[:], x1b[:])
            nc.scalar.square(sq2[:], x2b[:])

            # ---- column-moment partials: fold j 16->1, one ones-matmul ----
            # statcat = [s1_1 | s1_2 | s2_1 | s2_2] so downstream math runs
            # on [1,128]-wide slices (both inputs at once)
            statcat = sb.tile([128, 4 * F], bf16)
            for q, src in enumerate((x1b, x2b, sq1, sq2)):
                fa = sb.tile([128, 8, F], bf16, tag="folda", bufs=2)
                fb = sb.tile([128, 4, F], bf16, tag="foldb", bufs=2)
                fc = sb.tile([128, 2, F], bf16, tag="foldc", bufs=2)
                nc.vector.tensor_add(fa[:], src[:, 0:8, :], src[:, 8:16, :])
                nc.vector.tensor_add(fb[:], fa[:, 0:4, :], fa[:, 4:8, :])
                nc.vector.tensor_add(fc[:], fb[:, 0:2, :], fb[:, 2:4, :])
                nc.vector.tensor_add(
                    statcat[:, q * F:(q + 1) * F], fc[:, 0, :], fc[:, 1, :]
                )
            stat_ps = ps.tile([1, 4 * F], fp32, tag="psA")
            nc.tensor.matmul(stat_ps[:], ones_bf[:], statcat[:], start=True, stop=True)

            # ---- AllGather the per-core moment partials, fold on-core ----
            stat_sb = sb.tile([1, 4 * F], fp32)
            nc.vector.tensor_copy(stat_sb[:], stat_ps[:])
            ag1_in = dram.tile([1, 4 * F], fp32)
            ag1_out = dram.tile([N_CORES, 4 * F], fp32, addr_space="Shared")
            nc.sync.dma_start(ag1_in[:], stat_sb[:])
            nc.gpsimd.collective_compute(
                "AllGather", bypass, replica_groups=rg,
                ins=[ag1_in.opt()], outs=[ag1_out.opt()],
            )
            stats8 = sb.tile([N_CORES, 4 * F], fp32)
            nc.sync.dma_start(stats8[:], ag1_out[:])
            statg = ps.tile([1, 4 * F], fp32, tag="psB")
            nc.tensor.matmul(
                statg[:], ones_fc[0:N_CORES, :], stats8[:], start=True, stop=True
            )

            # ---- mu/inv_sd, both inputs at once on [1,128] slices ----
            # bsrc layout: [isd1 | isd2 | misd1 | misd2]
            bsrc = sb.tile([1, 4 * F], fp32)
            mu = sb.tile([1, 2 * F], fp32)
            mnn = sb.tile([1, 2 * F], fp32)
            var = sb.tile([1, 2 * F], fp32)
            sd = sb.tile([1, 2 * F], fp32)
            nc.vector.tensor_scalar(mu[:], statg[:, 0:2 * F], 1.0 / Nf, None, mult)
            nc.vector.scalar_tensor_tensor(
                mnn[:], mu[:], Nf / (Nf - 1.0), mu[:], mult, mult
            )
            nc.vector.scalar_tensor_tensor(
                var[:], statg[:, 2 * F:4 * F], 1.0 / (Nf - 1.0), mnn[:],
                mult, subtract,
            )
            nc.scalar.sqrt(sd[:], var[:])
            isd = bsrc[:, 0:2 * F]
            nc.vector.reciprocal(isd, sd[:])
            nc.vector.tensor_mul(bsrc[:, 2 * F:4 * F], mu[:], isd)

            # ---- broadcast across partitions via K=1 ones-matmuls ----
            # isd half first so standardization can start while misd runs
            bcp_i = ps.tile([128, 2 * F], fp32, tag="psC")
            bcp_m = ps.tile([128, 2 * F], fp32)
            nc.tensor.matmul(
                bcp_i[:], ones_fr[:, 0:128], bsrc[:, 0:2 * F], start=True, stop=True
            )
            nc.tensor.matmul(
                bcp_m[:], ones_fr[:, 0:128], bsrc[:, 2 * F:4 * F],
                start=True, stop=True,
            )
            bci = sb.tile([128, 2 * F], bf16)
            bcm = sb.tile([128, 2 * F], bf16)
            nc.vector.tensor_copy(bci[:], bcp_i[:])
            nc.vector.tensor_copy(bcm[:], bcp_m[:])
            ISD1 = bci[:, 0:F].unsqueeze(1).broadcast_to([128, J, F])
            ISD2 = bci[:, F:2 * F].unsqueeze(1).broadcast_to([128, J, F])
            MIS1 = bcm[:, 0:F].unsqueeze(1).broadcast_to([128, J, F])
            MIS2 = bcm[:, F:2 * F].unsqueeze(1).broadcast_to([128, J, F])

            # ---- standardize: xs = x*isd - mu*isd ----
            xs1 = sb.tile([128, J, F], bf16)
            xs2 = sb.tile([128, J, F], bf16)
            z1 = sb.tile([128, J, F], bf16, tag="zt", bufs=2)
            z2 = sb.tile([128, J, F], bf16, tag="zt", bufs=2)
            nc.vector.tensor_mul(z1[:], x1b[:], ISD1)
            nc.vector.tensor_sub(xs1[:], z1[:], MIS1)
            nc.vector.tensor_mul(z2[:], x2b[:], ISD2)
            nc.vector.tensor_sub(xs2[:], z2[:], MIS2)

            # ---- row norms^2 (squares on ACT), then 1/r^2 scaling ----
            sqs = sb.tile([128, J, F], bf16, tag="sqs", bufs=2)
            sqs2 = sb.tile([128, J, F], bf16, tag="sqs", bufs=2)
            r2_1 = sb.tile([128, J], fp32)
            r2_2 = sb.tile([128, J], fp32)
            nc.scalar.square(sqs[:], xs1[:])
            nc.vector.tensor_reduce(r2_1[:], sqs[:], AX, add)
            nc.scalar.square(sqs2[:], xs2[:])
            nc.vector.tensor_reduce(r2_2[:], sqs2[:], AX, add)
            w1 = sb.tile([128, J], fp32)
            w2 = sb.tile([128, J], fp32)
            nc.vector.reciprocal(w1[:], r2_1[:])
            nc.vector.reciprocal(w2[:], r2_2[:])
            w1b = sb.tile([128, J], bf16)
            w2b = sb.tile([128, J], bf16)
            nc.vector.tensor_copy(w1b[:], w1[:])
            nc.vector.tensor_copy(w2b[:], w2[:])

            # xw split in j-halves so Gram matmuls start after each half
            H = J // 2
            xw1a = sb.tile([128, H, F], bf16)
            xw1b = sb.tile([128, H, F], bf16)
            xw2a = sb.tile([128, H, F], bf16)
            xw2b = sb.tile([128, H, F], bf16)
            nc.vector.tensor_mul(
                xw1a[:], xs1[:, 0:H, :],
                w1b[:, 0:H].unsqueeze(2).broadcast_to([128, H, F]),
            )
            nc.vector.tensor_mul(
                xw1b[:], xs1[:, H:J, :],
                w1b[:, H:J].unsqueeze(2).broadcast_to([128, H, F]),
            )
            nc.vector.tensor_mul(
                xw2a[:], xs2[:, 0:H, :],
                w2b[:, 0:H].unsqueeze(2).broadcast_to([128, H, F]),
            )
            nc.vector.tensor_mul(
                xw2b[:], xs2[:, H:J, :],
                w2b[:, H:J].unsqueeze(2).broadcast_to([128, H, F]),
            )

            # ---- Gram partials A = xs1^T (xs1 * w1), B likewise ----
            gramA = ps.tile([F, F], fp32)
            gramB = ps.tile([F, F], fp32)
            for j in range(J):
                xw = (xw1a if j < H else xw1b)[:, j % H, :]
                nc.tensor.matmul(
                    gramA[:], xs1[:, j, :], xw,
                    start=(j == 0), stop=(j == J - 1),
                )
            for j in range(J):
                xw = (xw2a if j < H else xw2b)[:, j % H, :]
                nc.tensor.matmul(
                    gramB[:], xs2[:, j, :], xw,
                    start=(j == 0), stop=(j == J - 1),
                )

            # ---- diagonal terms: p_i = mnum_i / sqrt(r2_1 r2_2) ----
            mts = sb.tile([128, J, F], bf16, tag="sqs", bufs=2)
            mnum = sb.tile([128, J], fp32)
            nc.vector.tensor_mul(mts[:], xs1[:], xs2[:])
            nc.vector.tensor_reduce(mnum[:], mts[:], AX, add)
            qq = sb.tile([128, J], fp32)
            sqq = sb.tile([128, J], fp32)
            rq = sb.tile([128, J], fp32)
            pp = sb.tile([128, J], fp32)
            pcols = sb.tile([128, 2], fp32)
            psc = sb.tile([128, J], fp32)
            nc.vector.tensor_mul(qq[:], r2_1[:], r2_2[:])
            nc.scalar.sqrt(sqq[:], qq[:])
            nc.vector.reciprocal(rq[:], sqq[:])
            nc.vector.tensor_mul(pp[:], mnum[:], rq[:])
            nc.vector.tensor_reduce(pcols[:, 0:1], pp[:], AX, add)
            # psc = p^2, accum -> pcols[:,1]
            nc.vector.scalar_tensor_tensor(
                psc[:], pp[:], 1.0, pp[:], mult, mult, accum_out=pcols[:, 1:2]
            )
            sp_ps = ps.tile([1, 2], fp32, tag="psA")
            nc.tensor.matmul(sp_ps[:], ones_fc[:], pcols[:], start=True, stop=True)
            sp_sb = sb.tile([1, 2], fp32)
            nc.vector.tensor_copy(sp_sb[:], sp_ps[:])
            # broadcast (sp, spp) to 64 rows for the ReduceScatter payload
            spb_ps = ps.tile([F, 2], fp32, tag="psB")
            nc.tensor.matmul(
                spb_ps[:], ones_fr[:, 0:F], sp_sb[:], start=True, stop=True
            )

            # ---- ReduceScatter payload: [64 rows, A_f | B_f | sp | spp] ----
            C2 = 2 * F + 2
            ab_sb = sb.tile([F, C2], fp32)
            nc.vector.tensor_copy(ab_sb[:, 0:F], gramA[:])
            nc.vector.tensor_copy(ab_sb[:, F:2 * F], gramB[:])
            nc.vector.tensor_copy(ab_sb[:, 2 * F:C2], spb_ps[:])
            rs_in = dram.tile([F, C2], fp32)
            rs_out = dram.tile([F // N_CORES, C2], fp32)
            nc.sync.dma_start(rs_in[:], ab_sb[:])
            nc.gpsimd.collective_compute(
                "ReduceScatter", add, replica_groups=rg,
                ins=[rs_in.opt()], outs=[rs_out.opt()],
            )
            S = F // N_CORES  # 8 feature-rows of the global Grams per core
            rs_sb = sb.tile([S, C2], fp32)
            nc.sync.dma_start(rs_sb[:], rs_out[:])

            # ---- per-core partial loss (sp/spp are duplicated 8x -> /8) ----
            abm = sb.tile([S, F], fp32)
            abf = sb.tile([S, 1], fp32)
            nc.vector.tensor_mul(abm[:], rs_sb[:, 0:F], rs_sb[:, F:2 * F])
            nc.vector.tensor_reduce(abf[:], abm[:], AX, add)
            dot_ps = ps.tile([1, 1], fp32, tag="psC")
            nc.tensor.matmul(
                dot_ps[:], ones_fc[0:S, :], abf[:], start=True, stop=True
            )
            # t1 = (1 - sp/N)/8 ; t2 = (LAM/N)*dot_k - (LAM/N)*spp/8
            t1 = sb.tile([1, 1], fp32)
            t2 = sb.tile([1, 1], fp32)
            spp_s = sb.tile([1, 1], fp32)
            loss = sb.tile([1, 1], fp32)
            nc.vector.tensor_scalar(
                t1[:], rs_sb[0:1, 2 * F:2 * F + 1],
                -1.0 / (Nf * N_CORES), 1.0 / N_CORES, mult, add,
            )
            nc.vector.tensor_scalar(
                spp_s[:], rs_sb[0:1, 2 * F + 1:C2],
                LAM / (Nf * N_CORES), None, mult,
            )
            nc.vector.scalar_tensor_tensor(
                t2[:], dot_ps[:], LAM / Nf, spp_s[:], mult, subtract
            )
            nc.vector.tensor_add(loss[:], t1[:], t2[:])
            nc.sync.dma_start(out_d.ap(), loss[:])

    nc.compile()
    return nc


def _get_nc():
    if "nc" not in _BUILT:
        _BUILT["nc"] = _build_bass()
    return _BUILT["nc"]


def kernel(X1, X2):
    import ml_dtypes
    from concourse import bass_utils

    X1 = np.asarray(X1, dtype=np.float32).astype(ml_dtypes.bfloat16)
    X2 = np.asarray(X2, dtype=np.float32).astype(ml_dtypes.bfloat16)
    assert X1.shape == (N_TOTAL, F) and X2.shape == (N_TOTAL, F)

    nc = _get_nc()
    in_maps = [
        {
            "x1": np.ascontiguousarray(X1[k * ROWS:(k + 1) * ROWS]),
            "x2": np.ascontiguousarray(X2[k * ROWS:(k + 1) * ROWS]),
        }
        for k in range(N_CORES)
    ]
    res = bass_utils.run_bass_kernel_spmd(nc, in_maps, list(range(N_CORES)))
    partials = [np.float64(r["out"][0, 0]) for r in res.results]
    return np.float32(sum(partials))


# revision 8
# speedup vs baseline: 1.3665x; 1.3665x over previous
"""BarlowTwinsLoss on 8 Trainium2 NeuronCores.

Math: with xs = standardize(X1), ys = standardize(X2) (per-feature batch
stats, ddof=1), C = cos-sim matrix of rows: C[i,j] = u_i . v_j where
u_i = xs_i/|xs_i|, v_j = ys_j/|ys_j|.  The loss only needs
  inv_term = (N - sum_i C_ii)/N
  red_term = LAM/N * (sum_ij C_ij^2 - sum_i C_ii^2)
and sum_ij C_ij^2 = <U^T U, V^T V>_F, which collapses the O(N^2 F) problem
to O(N F^2): two [F,F] Gram matrices.

Distribution: rows sharded 8 ways, shipped as one bf16 buffer per core
(smaller/fewer H2D transfers -> less cross-core start skew, which is what
the first collective's rendezvous waits on).  Per core: partial column
moments -> AllGather + on-core fold (global mu/sd) -> standardize local
rows -> local Gram partials A_c, B_c [64,64] + diag partials ->
ReduceScatter so core k holds 8 feature-rows of the global A and B ->
per-core partial scalar loss.  Host sums the 8 partial losses (the
"all-reduce the scalar partial losses" step of the sharding hint).
"""

import numpy as np

N_CORES = 8
N_TOTAL = 16384
F = 64
ROWS = N_TOTAL // N_CORES  # 2048 rows per core
J = 16                     # free-dim row-chunks per partition: 128 * 16 = 2048
LAM = 0.2

_BUILT = {}


def _build_bass():
    import concourse.bacc as bacc
    import concourse.mybir as mybir
    import concourse.tile as tile

    fp32 = mybir.dt.float32
    bf16 = mybir.dt.bfloat16
    mult = mybir.AluOpType.mult
    add = mybir.AluOpType.add
    subtract = mybir.AluOpType.subtract
    bypass = mybir.AluOpType.bypass
    AX = mybir.AxisListType.X

    nc = bacc.Bacc(
        "TRN2", target_bir_lowering=False, debug=False, num_devices=N_CORES
    )

    # one input buffer per core: rows 0:2048 = X1 shard, 2048:4096 = X2 shard
    x_d = nc.dram_tensor("x", [2 * ROWS, F], bf16, kind="ExternalInput")
    out_d = nc.dram_tensor("out", [1, 1], fp32, kind="ExternalOutput")

    rg = [list(range(N_CORES))]
    Nf = float(N_TOTAL)

    with tile.TileContext(nc) as tc:
        with (
            tc.tile_pool(name="sb", bufs=1) as sb,
            tc.tile_pool(name="ps", bufs=1, space="PSUM") as ps,
            tc.tile_pool(name="dram", bufs=1, space="DRAM") as dram,
        ):
            # ---- constants ----
            ones_bf = sb.tile([128, 1], bf16)
            ones_fr = sb.tile([1, 128], fp32)   # row of ones (K=1 bcast matmuls)
            ones_fc = sb.tile([128, 1], fp32)   # column of ones (partition folds)
            nc.vector.memset(ones_bf[:], 1.0)
            nc.vector.memset(ones_fr[:], 1.0)
            nc.vector.memset(ones_fc[:], 1.0)

            # ---- load inputs: [2048,64] -> [128 partitions, 16 chunks, 64] ----
            # partition p holds rows p*16 .. p*16+15 (2KB contiguous per partition)
            x1b = sb.tile([128, J, F], bf16)
            x2b = sb.tile([128, J, F], bf16)
            xap = x_d.ap().rearrange("(h p j) f -> h p j f", h=2, p=128)
            nc.sync.dma_start(x1b[:], xap[0])
            nc.sync.dma_start(x2b[:], xap[1])

            # ---- squares on ACT ----
            sq1 = sb.tile([128, J, F], bf16)
            sq2 = sb.tile([128, J, F], bf16)
            nc.scalar.square(sq1[:], x1b[:])
            nc.scalar.square(sq2[:], x2b[:])

            # ---- column-moment partials: fold j 16->1, one ones-matmul ----
            # statcat = [s1_1 | s1_2 | s2_1 | s2_2] so downstream math runs
            # on [1,128]-wide slices (both inputs at once)
            statcat = sb.tile([128, 4 * F], bf16)
            for q, src in enumerate((x1b, x2b, sq1, sq2)):
                fa = sb.tile([128, 8, F], bf16, tag="folda", bufs=2)
                fb = sb.tile([128, 4, F], bf16, tag="foldb", bufs=2)
                fc = sb.tile([128, 2, F], bf16, tag="foldc", bufs=2)
                nc.vector.tensor_add(fa[:], src[:, 0:8, :], src[:, 8:16, :])
                nc.vector.tensor_add(fb[:], fa[:, 0:4, :], fa[:, 4:8, :])
                nc.vector.tensor_add(fc[:], fb[:, 0:2, :], fb[:, 2:4, :])
                nc.vector.tensor_add(
                    statcat[:, q * F:(q + 1) * F], fc[:, 0, :], fc[:, 1, :]
                )
            stat_ps = ps.tile([1, 4 * F], fp32, tag="psA")
            nc.tensor.matmul(stat_ps[:], ones_bf[:], statcat[:], start=True, stop=True)

            # ---- AllGather the per-core moment partials, fold on-core ----
            stat_sb = sb.tile([1, 4 * F], fp32)
            nc.vector.tensor_copy(stat_sb[:], stat_ps[:])
            ag1_in = dram.tile([1, 4 * F], fp32)
            ag1_out = dram.tile([N_CORES, 4 * F], fp32, addr_space="Shared")
            nc.sync.dma_start(ag1_in[:], stat_sb[:])
            nc.gpsimd.collective_compute(
                "AllGather", bypass, replica_groups=rg,
                ins=[ag1_in.opt()], outs=[ag1_out.opt()],
            )
            stats8 = sb.tile([N_CORES, 4 * F], fp32)
            nc.sync.dma_start(stats8[:], ag1_out[:])
            statg = ps.tile([1, 4 * F], fp32, tag="psB")
            nc.tensor.matmul(
                statg[:], ones_fc[0:N_CORES, :], stats8[:], start=True, stop=True
            )

            # ---- inv_sd and mu*inv_sd, both inputs at once on [1,128] ----
            # bsrc layout: [isd1 | isd2 | misd1 | misd2]
            bsrc = sb.tile([1, 4 * F], fp32)
            mnn = sb.tile([1, 2 * F], fp32)
            var = sb.tile([1, 2 * F], fp32)
            sd = sb.tile([1, 2 * F], fp32)
            sg = sb.tile([1, 4 * F], fp32)
            nc.vector.tensor_copy(sg[:], statg[:])
            s1g = sg[:, 0:2 * F]
            s2g = sg[:, 2 * F:4 * F]
            # mnn = s1^2/(N(N-1));  var = s2/(N-1) - mnn
            nc.vector.scalar_tensor_tensor(
                mnn[:], s1g, 1.0 / (Nf * (Nf - 1.0)), s1g, mult, mult
            )
            nc.vector.scalar_tensor_tensor(
                var[:], s2g, 1.0 / (Nf - 1.0), mnn[:], mult, subtract
            )
            nc.scalar.sqrt(sd[:], var[:])
            isd = bsrc[:, 0:2 * F]
            nc.vector.reciprocal_approx_fast(isd, sd[:])
            # misd = (s1/N) * isd
            nc.vector.scalar_tensor_tensor(
                bsrc[:, 2 * F:4 * F], s1g, 1.0 / Nf, isd, mult, mult
            )

            # ---- broadcast across partitions via K=1 ones-matmuls ----
            # isd half first so standardization can start while misd runs
            bcp_i = ps.tile([128, 2 * F], fp32, tag="psC")
            bcp_m = ps.tile([128, 2 * F], fp32)
            nc.tensor.matmul(
                bcp_i[:], ones_fr[:, 0:128], bsrc[:, 0:2 * F], start=True, stop=True
            )
            nc.tensor.matmul(
                bcp_m[:], ones_fr[:, 0:128], bsrc[:, 2 * F:4 * F],
                start=True, stop=True,
            )
            bci = sb.tile([128, 2 * F], bf16)
            bcm = sb.tile([128, 2 * F], bf16)
            nc.vector.tensor_copy(bci[:], bcp_i[:])
            nc.vector.tensor_copy(bcm[:], bcp_m[:])
            ISD1 = bci[:, 0:F].unsqueeze(1).broadcast_to([128, J, F])
            ISD2 = bci[:, F:2 * F].unsqueeze(1).broadcast_to([128, J, F])
            MIS1 = bcm[:, 0:F].unsqueeze(1).broadcast_to([128, J, F])
            MIS2 = bcm[:, F:2 * F].unsqueeze(1).broadcast_to([128, J, F])

            # ---- standardize: xs = x*isd - mu*isd ----
            xs1 = sb.tile([128, J, F], bf16)
            xs2 = sb.tile([128, J, F], bf16)
            z1 = sb.tile([128, J, F], bf16, tag="zt", bufs=2)
            z2 = sb.tile([128, J, F], bf16, tag="zt", bufs=2)
            nc.vector.tensor_mul(z1[:], x1b[:], ISD1)
            nc.vector.tensor_sub(xs1[:], z1[:], MIS1)
            nc.vector.tensor_mul(z2[:], x2b[:], ISD2)
            nc.vector.tensor_sub(xs2[:], z2[:], MIS2)

            # ---- per-input: row norm^2 -> w=1/r^2 -> xw, then Gram ----
            H = J // 2
            sqs = sb.tile([128, J, F], bf16, tag="sqs", bufs=2)
            sqs2 = sb.tile([128, J, F], bf16, tag="sqs", bufs=2)
            r2_1 = sb.tile([128, J], fp32)
            r2_2 = sb.tile([128, J], fp32)
            w1 = sb.tile([128, J], fp32)
            w2 = sb.tile([128, J], fp32)
            w1b = sb.tile([128, J], bf16)
            w2b = sb.tile([128, J], bf16)
            xw1a = sb.tile([128, H, F], bf16)
            xw1b = sb.tile([128, H, F], bf16)
            xw2a = sb.tile([128, H, F], bf16)
            xw2b = sb.tile([128, H, F], bf16)

            nc.scalar.square(sqs[:], xs1[:])
            nc.vector.tensor_reduce(r2_1[:], sqs[:], AX, add)
            nc.vector.reciprocal_approx_fast(w1[:], r2_1[:])
            nc.vector.tensor_copy(w1b[:], w1[:])
            nc.vector.tensor_mul(
                xw1a[:], xs1[:, 0:H, :],
                w1b[:, 0:H].unsqueeze(2).broadcast_to([128, H, F]),
            )
            nc.vector.tensor_mul(
                xw1b[:], xs1[:, H:J, :],
                w1b[:, H:J].unsqueeze(2).broadcast_to([128, H, F]),
            )
            nc.scalar.square(sqs2[:], xs2[:])
            nc.vector.tensor_reduce(r2_2[:], sqs2[:], AX, add)
            nc.vector.reciprocal_approx_fast(w2[:], r2_2[:])
            nc.vector.tensor_copy(w2b[:], w2[:])
            nc.vector.tensor_mul(
                xw2a[:], xs2[:, 0:H, :],
                w2b[:, 0:H].unsqueeze(2).broadcast_to([128, H, F]),
            )
            nc.vector.tensor_mul(
                xw2b[:], xs2[:, H:J, :],
                w2b[:, H:J].unsqueeze(2).broadcast_to([128, H, F]),
            )

            # ---- Gram partials A = xs1^T (xs1 * w1), B likewise ----
            gramA = ps.tile([F, F], fp32)
            gramB = ps.tile([F, F], fp32)
            for j in range(J):
                xw = (xw1a if j < H else xw1b)[:, j % H, :]
                nc.tensor.matmul(
                    gramA[:], xs1[:, j, :], xw,
                    start=(j == 0), stop=(j == J - 1),
                )
            for j in range(J):
                xw = (xw2a if j < H else xw2b)[:, j % H, :]
                nc.tensor.matmul(
                    gramB[:], xs2[:, j, :], xw,
                    start=(j == 0), stop=(j == J - 1),
                )

            # ---- diagonal terms: p_i = mnum_i / sqrt(r2_1 r2_2) ----
            mts = sb.tile([128, J, F], bf16, tag="sqs", bufs=2)
            mnum = sb.tile([128, J], fp32)
            nc.vector.tensor_mul(mts[:], xs1[:], xs2[:])
            nc.vector.tensor_reduce(mnum[:], mts[:], AX, add)
            qq = sb.tile([128, J], fp32)
            sqq = sb.tile([128, J], fp32)
            rq = sb.tile([128, J], fp32)
            pp = sb.tile([128, J], fp32)
            pcols = sb.tile([128, 2], fp32)
            psc = sb.tile([128, J], fp32)
            nc.vector.tensor_mul(qq[:], r2_1[:], r2_2[:])
            nc.scalar.sqrt(sqq[:], qq[:])
            nc.vector.reciprocal_approx_fast(rq[:], sqq[:])
            nc.vector.tensor_mul(pp[:], mnum[:], rq[:])
            nc.vector.tensor_reduce(pcols[:, 0:1], pp[:], AX, add)
            # psc = p^2, accum -> pcols[:,1]
            nc.vector.scalar_tensor_tensor(
                psc[:], pp[:], 1.0, pp[:], mult, mult, accum_out=pcols[:, 1:2]
            )
            sp_ps = ps.tile([1, 2], fp32, tag="psA")
            nc.tensor.matmul(sp_ps[:], ones_fc[:], pcols[:], start=True, stop=True)
            sp_sb = sb.tile([1, 2], fp32)
            nc.vector.tensor_copy(sp_sb[:], sp_ps[:])
            # broadcast (sp, spp) to 64 rows for the ReduceScatter payload
            spb_ps = ps.tile([F, 2], fp32, tag="psB")
            nc.tensor.matmul(
                spb_ps[:], ones_fr[:, 0:F], sp_sb[:], start=True, stop=True
            )

            # ---- ReduceScatter payload: [64 rows, A_f | B_f | sp | spp] ----
            C2 = 2 * F + 2
            ab_sb = sb.tile([F, C2], fp32)
            nc.vector.tensor_copy(ab_sb[:, 0:F], gramA[:])
            nc.vector.tensor_copy(ab_sb[:, F:2 * F], gramB[:])
            nc.vector.tensor_copy(ab_sb[:, 2 * F:C2], spb_ps[:])
            rs_in = dram.tile([F, C2], fp32)
            rs_out = dram.tile([F // N_CORES, C2], fp32)
            nc.sync.dma_start(rs_in[:], ab_sb[:])
            nc.gpsimd.collective_compute(
                "ReduceScatter", add, replica_groups=rg,
                ins=[rs_in.opt()], outs=[rs_out.opt()],
            )
            S = F // N_CORES  # 8 feature-rows of the global Grams per core
            rs_sb = sb.tile([S, C2], fp32)
            nc.sync.dma_start(rs_sb[:], rs_out[:])

            # ---- per-core partial loss (sp/spp are duplicated 8x -> /8) ----
            abm = sb.tile([S, F], fp32)
            abf = sb.tile([S, 1], fp32)
            nc.vector.tensor_mul(abm[:], rs_sb[:, 0:F], rs_sb[:, F:2 * F])
            nc.vector.tensor_reduce(abf[:], abm[:], AX, add)
            dot_ps = ps.tile([1, 1], fp32, tag="psC")
            nc.tensor.matmul(
                dot_ps[:], ones_fc[0:S, :], abf[:], start=True, stop=True
            )
            # t1 = (1 - sp/N)/8 ; t2 = (LAM/N)*dot_k - (LAM/N)*spp/8
            t1 = sb.tile([1, 1], fp32)
            t2 = sb.tile([1, 1], fp32)
            spp_s = sb.tile([1, 1], fp32)
            loss = sb.tile([1, 1], fp32)
            nc.vector.tensor_scalar(
                t1[:], rs_sb[0:1, 2 * F:2 * F + 1],
                -1.0 / (Nf * N_CORES), 1.0 / N_CORES, mult, add,
            )
            nc.vector.tensor_scalar(
                spp_s[:], rs_sb[0:1, 2 * F + 1:C2],
                LAM / (Nf * N_CORES), None, mult,
            )
            nc.vector.scalar_tensor_tensor(
                t2[:], dot_ps[:], LAM / Nf, spp_s[:], mult, subtract
            )
            nc.vector.tensor_add(loss[:], t1[:], t2[:])
            nc.sync.dma_start(out_d.ap(), loss[:])

    nc.compile()
    return nc


def _get_nc():
    if "nc" not in _BUILT:
        _BUILT["nc"] = _build_bass()
    return _BUILT["nc"]


def _get_runner():
    """Cached jitted SPMD executor.

    Same lowering as concourse.bass2jax.run_bass_via_pjrt, but (a) the
    jitted callable is built once and reused, and (b) inputs are pre-placed
    with jax.device_put under an explicit sharding so per-core H2D staging
    overlaps instead of serializing inside the execute (the cross-core
    start skew is what the first collective's rendezvous waits on).
    """
    if "runner" in _BUILT:
        return _BUILT["runner"]

    import jax
    import concourse.mybir as mybir
    from concourse.bass2jax import (
        _bass_exec_p,
        install_neuronx_cc_hook,
        partition_id_tensor,
    )
    from jax.experimental.shard_map import shard_map
    from jax.sharding import Mesh, NamedSharding, PartitionSpec

    nc = _get_nc()
    install_neuronx_cc_hook()

    partition_name = (
        nc.partition_id_tensor.name if nc.partition_id_tensor else None
    )
    in_names, out_names, out_avals, zero_outs = [], [], [], []
    for alloc in nc.m.functions[0].allocations:
        if not isinstance(alloc, mybir.MemoryLocationSet):
            continue
        name = alloc.memorylocations[0].name
        if alloc.kind == "ExternalInput":
            if name != partition_name:
                in_names.append(name)
        elif alloc.kind == "ExternalOutput":
            shape = tuple(alloc.tensor_shape)
            dtype = mybir.dt.np(alloc.dtype)
            out_names.append(name)
            out_avals.append(jax.core.ShapedArray(shape, dtype))
            zero_outs.append(np.zeros(shape, dtype))
    n_params = len(in_names)
    all_names = in_names + out_names
    if partition_name is not None:
        all_names = all_names + [partition_name]
    donate = tuple(range(n_params, n_params + len(out_names)))

    def _body(*args):
        operands = list(args)
        if partition_name is not None:
            operands.append(partition_id_tensor())
        outs = _bass_exec_p.bind(
            *operands,
            out_avals=tuple(out_avals),
            in_names=tuple(all_names),
            out_names=tuple(out_names),
            lowering_input_output_aliases=(),
            sim_require_finite=True,
            sim_require_nnan=True,
            nc=nc,
        )
        return tuple(outs)

    devices = jax.devices()[:N_CORES]
    mesh = Mesh(np.asarray(devices), ("core",))
    spec = NamedSharding(mesh, PartitionSpec("core"))
    n_all = n_params + len(zero_outs)
    sharded = jax.jit(
        shard_map(
            _body, mesh=mesh,
            in_specs=(PartitionSpec("core"),) * n_all,
            out_specs=(PartitionSpec("core"),) * len(out_names),
            check_rep=False,
        ),
        donate_argnums=donate,
        keep_unused=True,
    )

    def run(in_maps):
        concat_in = [
            np.concatenate([np.asarray(m[name]) for m in in_maps], axis=0)
            for name in in_names
        ]
        concat_zero = [
            np.zeros((N_CORES * z.shape[0], *z.shape[1:]), z.dtype)
            for z in zero_outs
        ]
        args = [jax.device_put(a, spec) for a in concat_in + concat_zero]
        out_arrs = sharded(*args)
        return [
            {
                name: np.asarray(out_arrs[i]).reshape(
                    N_CORES, *out_avals[i].shape
                )[c]
                for i, name in enumerate(out_names)
            }
            for c in range(N_CORES)
        ]

    _BUILT["runner"] = run
    return run


def kernel(X1, X2):
    import ml_dtypes

    X1 = np.asarray(X1, dtype=np.float32).astype(ml_dtypes.bfloat16)
    X2 = np.asarray(X2, dtype=np.float32).astype(ml_dtypes.bfloat16)
    assert X1.shape == (N_TOTAL, F) and X2.shape == (N_TOTAL, F)

    run = _get_runner()
    in_maps = [
        {
            "x": np.concatenate(
                [X1[k * ROWS:(k + 1) * ROWS], X2[k * ROWS:(k + 1) * ROWS]]
            )
        }
        for k in range(N_CORES)
    ]
    results = run(in_maps)
    partials = [np.float64(r["out"][0, 0]) for r in results]
    return np.float32(sum(partials))


# revision 11
# speedup vs baseline: 1.4868x; 1.0881x over previous
"""BarlowTwinsLoss on 8 Trainium2 NeuronCores.

Math: with xs = standardize(X1), ys = standardize(X2) (per-feature batch
stats, ddof=1), C = cos-sim matrix of rows: C[i,j] = u_i . v_j where
u_i = xs_i/|xs_i|, v_j = ys_j/|ys_j|.  The loss only needs
  inv_term = (N - sum_i C_ii)/N
  red_term = LAM/N * (sum_ij C_ij^2 - sum_i C_ii^2)
and sum_ij C_ij^2 = <U^T U, V^T V>_F, which collapses the O(N^2 F) problem
to O(N F^2): two [F,F] Gram matrices.

Distribution: rows sharded 8 ways, shipped as one bf16 buffer per core
(smaller/fewer H2D transfers -> less cross-core start skew, which is what
the first collective's rendezvous waits on).  Per core: partial column
moments -> AllGather + on-core fold (global mu/sd) -> standardize local
rows -> local Gram partials A_c, B_c [64,64] + diag partials ->
ReduceScatter so core k holds 8 feature-rows of the global A and B ->
per-core partial scalar loss.  Host sums the 8 partial losses (the
"all-reduce the scalar partial losses" step of the sharding hint).
"""

import numpy as np

N_CORES = 8
N_TOTAL = 16384
F = 64
ROWS = N_TOTAL // N_CORES  # 2048 rows per core
J = 16                     # free-dim row-chunks per partition: 128 * 16 = 2048
LAM = 0.2

_BUILT = {}


def _build_bass():
    import concourse.bacc as bacc
    import concourse.mybir as mybir
    import concourse.tile as tile

    fp32 = mybir.dt.float32
    bf16 = mybir.dt.bfloat16
    mult = mybir.AluOpType.mult
    add = mybir.AluOpType.add
    subtract = mybir.AluOpType.subtract
    bypass = mybir.AluOpType.bypass
    AX = mybir.AxisListType.X

    nc = bacc.Bacc(
        "TRN2", target_bir_lowering=False, debug=False, num_devices=N_CORES,
        enable_asserts=False,
    )

    # one input buffer per core: rows 0:2048 = X1 shard, 2048:4096 = X2 shard
    x_d = nc.dram_tensor("x", [2 * ROWS, F], bf16, kind="ExternalInput")
    out_d = nc.dram_tensor("out", [1, 3], fp32, kind="ExternalOutput")

    rg = [list(range(N_CORES))]
    Nf = float(N_TOTAL)

    with tile.TileContext(nc) as tc:
        with (
            tc.tile_pool(name="sb", bufs=1) as sb,
            tc.tile_pool(name="ps", bufs=1, space="PSUM") as ps,
            tc.tile_pool(name="dram", bufs=1, space="DRAM") as dram,
        ):
            # ---- constants ----
            ones_bf = sb.tile([128, 1], bf16)
            ones_fr = sb.tile([1, 128], fp32)   # row of ones (K=1 bcast matmuls)
            ones_fc = sb.tile([128, 1], fp32)   # column of ones (partition folds)
            nc.vector.memset(ones_bf[:], 1.0)
            nc.vector.memset(ones_fr[:], 1.0)
            nc.vector.memset(ones_fc[:], 1.0)
            actwarm = sb.tile([1, 1], fp32)
            nc.scalar.sqrt(actwarm[:], ones_fr[0:1, 0:1])

            # ---- load inputs: [2048,64] -> [128 partitions, 16 chunks, 64] ----
            # partition p holds rows p*16 .. p*16+15 (2KB contiguous per partition)
            x1b = sb.tile([128, J, F], bf16)
            x2b = sb.tile([128, J, F], bf16)
            xap = x_d.ap().rearrange("(h p j) f -> h p j f", h=2, p=128)
            nc.sync.dma_start(x1b[:], xap[0])
            nc.scalar.dma_start(x2b[:], xap[1])

            # ---- squares on ACT ----
            sq1 = sb.tile([128, J, F], bf16)
            sq2 = sb.tile([128, J, F], bf16)
            nc.vector.tensor_mul(sq1[:], x1b[:], x1b[:])
            nc.vector.tensor_mul(sq2[:], x2b[:], x2b[:])

            # ---- column-moment partials: fold j 16->1, one ones-matmul ----
            # statcat = [s1_1 | s1_2 | s2_1 | s2_2] so downstream math runs
            # on [1,128]-wide slices (both inputs at once)
            statcat = sb.tile([128, 4 * F], bf16)
            for q, src in enumerate((x1b, x2b, sq1, sq2)):
                fa = sb.tile([128, 8, F], bf16, tag="folda", bufs=2)
                fb = sb.tile([128, 4, F], bf16, tag="foldb", bufs=2)
                fc = sb.tile([128, 2, F], bf16, tag="foldc", bufs=2)
                nc.vector.tensor_add(fa[:], src[:, 0:8, :], src[:, 8:16, :])
                nc.vector.tensor_add(fb[:], fa[:, 0:4, :], fa[:, 4:8, :])
                nc.vector.tensor_add(fc[:], fb[:, 0:2, :], fb[:, 2:4, :])
                nc.vector.tensor_add(
                    statcat[:, q * F:(q + 1) * F], fc[:, 0, :], fc[:, 1, :]
                )
            stat_ps = ps.tile([1, 4 * F], fp32, tag="psA")
            nc.tensor.matmul(stat_ps[:], ones_bf[:], statcat[:], start=True, stop=True)

            # ---- AllGather the per-core moment partials, fold on-core ----
            stat_sb = sb.tile([1, 4 * F], fp32)
            nc.vector.tensor_copy(stat_sb[:], stat_ps[:])
            ag1_in = dram.tile([1, 4 * F], fp32)
            ag1_out = dram.tile([N_CORES, 4 * F], fp32, addr_space="Shared")
            nc.sync.dma_start(ag1_in[:], stat_sb[:])
            nc.gpsimd.collective_compute(
                "AllGather", bypass, replica_groups=rg,
                ins=[ag1_in.opt()], outs=[ag1_out.opt()],
            )
            stats8 = sb.tile([N_CORES, 4 * F], fp32)
            nc.sync.dma_start(stats8[:], ag1_out[:])
            statg = ps.tile([1, 4 * F], fp32, tag="psB")
            nc.tensor.matmul(
                statg[:], ones_fc[0:N_CORES, :], stats8[:], start=True, stop=True
            )

            # ---- inv_sd and mu*inv_sd, both inputs at once on [1,128] ----
            # bsrc layout: [isd1 | isd2 | misd1 | misd2]
            bsrc = sb.tile([1, 4 * F], fp32)
            mnn = sb.tile([1, 2 * F], fp32)
            var = sb.tile([1, 2 * F], fp32)
            sd = sb.tile([1, 2 * F], fp32)
            # mnn = (s1/sqrt(N(N-1)))^2 on ACT, reading statg straight from PSUM
            nc.scalar.activation(
                mnn[:], statg[:, 0:2 * F], mybir.ActivationFunctionType.Square,
                bias=0.0, scale=(Nf * (Nf - 1.0)) ** -0.5,
            )
            # var = s2/(N-1) - mnn  (single PSUM operand is allowed)
            nc.vector.scalar_tensor_tensor(
                var[:], statg[:, 2 * F:4 * F], 1.0 / (Nf - 1.0), mnn[:],
                mult, subtract,
            )
            nc.scalar.sqrt(sd[:], var[:])
            isd = bsrc[:, 0:2 * F]
            nc.vector.reciprocal_approx_fast(isd, sd[:])
            # misd = (s1/N) * isd
            nc.vector.scalar_tensor_tensor(
                bsrc[:, 2 * F:4 * F], statg[:, 0:2 * F], 1.0 / Nf, isd, mult, mult
            )

            # ---- broadcast across partitions via K=1 ones-matmuls ----
            # isd half first so standardization can start while misd runs
            bcp_i = ps.tile([128, 2 * F], fp32, tag="psC")
            bcp_m = ps.tile([128, 2 * F], fp32)
            nc.tensor.matmul(
                bcp_i[:], ones_fr[:, 0:128], bsrc[:, 0:2 * F], start=True, stop=True
            )
            nc.tensor.matmul(
                bcp_m[:], ones_fr[:, 0:128], bsrc[:, 2 * F:4 * F],
                start=True, stop=True,
            )
            bci = sb.tile([128, 2 * F], bf16)
            bcm = sb.tile([128, 2 * F], bf16)
            nc.vector.tensor_copy(bci[:], bcp_i[:])
            nc.scalar.copy(bcm[:], bcp_m[:])
            ISD1 = bci[:, 0:F].unsqueeze(1).broadcast_to([128, J, F])
            ISD2 = bci[:, F:2 * F].unsqueeze(1).broadcast_to([128, J, F])
            MIS1 = bcm[:, 0:F].unsqueeze(1).broadcast_to([128, J, F])
            MIS2 = bcm[:, F:2 * F].unsqueeze(1).broadcast_to([128, J, F])

            # ---- standardize: xs = x*isd - mu*isd ----
            xs1 = sb.tile([128, J, F], bf16)
            xs2 = sb.tile([128, J, F], bf16)
            z1 = sb.tile([128, J, F], bf16, tag="zt", bufs=2)
            z2 = sb.tile([128, J, F], bf16, tag="zt", bufs=2)
            nc.vector.tensor_mul(z1[:], x1b[:], ISD1)
            nc.vector.tensor_sub(xs1[:], z1[:], MIS1)
            nc.vector.tensor_mul(z2[:], x2b[:], ISD2)
            nc.vector.tensor_sub(xs2[:], z2[:], MIS2)

            # ---- per-input: row norm^2 -> w=1/r^2 -> xw, then Gram ----
            H = J // 2
            sqs = sb.tile([128, J, F], bf16, tag="sqs", bufs=2)
            sqs2 = sb.tile([128, J, F], bf16, tag="sqs", bufs=2)
            r2_1 = sb.tile([128, J], fp32)
            r2_2 = sb.tile([128, J], fp32)
            w1 = sb.tile([128, J], fp32)
            w2 = sb.tile([128, J], fp32)
            w1b = sb.tile([128, J], bf16)
            w2b = sb.tile([128, J], bf16)
            xw1a = sb.tile([128, H, F], bf16)
            xw1b = sb.tile([128, H, F], bf16)
            xw2a = sb.tile([128, H, F], bf16)
            xw2b = sb.tile([128, H, F], bf16)

            nc.scalar.square(sqs[:], xs1[:])
            nc.vector.tensor_reduce(r2_1[:], sqs[:], AX, add)
            nc.vector.reciprocal_approx_fast(w1[:], r2_1[:])
            nc.vector.tensor_copy(w1b[:], w1[:])
            nc.vector.tensor_mul(
                xw1a[:], xs1[:, 0:H, :],
                w1b[:, 0:H].unsqueeze(2).broadcast_to([128, H, F]),
            )
            nc.vector.tensor_mul(
                xw1b[:], xs1[:, H:J, :],
                w1b[:, H:J].unsqueeze(2).broadcast_to([128, H, F]),
            )
            nc.scalar.square(sqs2[:], xs2[:])
            nc.vector.tensor_reduce(r2_2[:], sqs2[:], AX, add)
            nc.vector.reciprocal_approx_fast(w2[:], r2_2[:])
            nc.vector.tensor_copy(w2b[:], w2[:])
            nc.vector.tensor_mul(
                xw2a[:], xs2[:, 0:H, :],
                w2b[:, 0:H].unsqueeze(2).broadcast_to([128, H, F]),
            )
            last_xw = nc.vector.tensor_mul(
                xw2b[:], xs2[:, H:J, :],
                w2b[:, H:J].unsqueeze(2).broadcast_to([128, H, F]),
            )

            # ---- Gram partials A = xs1^T (xs1 * w1), B likewise ----
            gramA = ps.tile([F, F], fp32)
            gramB = ps.tile([F, F], fp32)
            for j in range(J):
                xw = (xw1a if j < H else xw1b)[:, j % H, :]
                nc.tensor.matmul(
                    gramA[:], xs1[:, j, :], xw,
                    start=(j == 0), stop=(j == J - 1),
                )
            for j in range(J):
                xw = (xw2a if j < H else xw2b)[:, j % H, :]
                nc.tensor.matmul(
                    gramB[:], xs2[:, j, :], xw,
                    start=(j == 0), stop=(j == J - 1),
                )

            # ---- diagonal terms: p_i = mnum_i / sqrt(r2_1 r2_2) ----
            mts = sb.tile([128, J, F], bf16, tag="sqs", bufs=2)
            mnum = sb.tile([128, J], fp32)
            nc.vector.tensor_mul(mts[:], xs1[:], xs2[:])
            mnum_red = nc.vector.tensor_reduce(mnum[:], mts[:], AX, add)
            from concourse.tile_rust import add_dep_helper
            add_dep_helper(
                mnum_red.ins, last_xw.ins, sync=False,
                reason="keep diag path off the xw/Gram critical path",
            )
            qq = sb.tile([128, J], fp32)
            sqq = sb.tile([128, J], fp32)
            rq = sb.tile([128, J], fp32)
            pp = sb.tile([128, J], fp32)
            pcols = sb.tile([128, 2], fp32)
            psc = sb.tile([128, J], fp32)
            nc.vector.tensor_mul(qq[:], r2_1[:], r2_2[:])
            nc.scalar.sqrt(sqq[:], qq[:])
            nc.vector.reciprocal_approx_fast(rq[:], sqq[:])
            nc.vector.tensor_mul(pp[:], mnum[:], rq[:])
            nc.vector.tensor_reduce(pcols[:, 0:1], pp[:], AX, add)
            # psc = p^2, accum -> pcols[:,1]
            nc.vector.scalar_tensor_tensor(
                psc[:], pp[:], 1.0, pp[:], mult, mult, accum_out=pcols[:, 1:2]
            )
            sp_ps = ps.tile([1, 2], fp32, tag="psA")
            nc.tensor.matmul(sp_ps[:], ones_fc[:], pcols[:], start=True, stop=True)

            # ---- ReduceScatter payload: [64 rows, A_f | B_f] ----
            C2 = 2 * F
            ab_sb = sb.tile([F, C2], fp32)
            nc.vector.tensor_copy(ab_sb[:, 0:F], gramA[:])
            nc.vector.tensor_copy(ab_sb[:, F:2 * F], gramB[:])
            rs_in = dram.tile([F, C2], fp32)
            rs_out = dram.tile([F // N_CORES, C2], fp32)
            nc.scalar.dma_start(rs_in[:, 0:F], ab_sb[:, 0:F])
            nc.sync.dma_start(rs_in[:, F:C2], ab_sb[:, F:C2])
            nc.gpsimd.collective_compute(
                "ReduceScatter", add, replica_groups=rg,
                ins=[rs_in.opt()], outs=[rs_out.opt()],
            )
            S = F // N_CORES  # 8 feature-rows of the global Grams per core
            rs_sb = sb.tile([S, C2], fp32)
            nc.sync.dma_start(rs_sb[:], rs_out[:])

            # ---- per-core partials: dot over this core's 8 feature-rows ----
            abm = sb.tile([S, F], fp32)
            abf = sb.tile([S, 1], fp32)
            nc.vector.tensor_mul(abm[:], rs_sb[:, 0:F], rs_sb[:, F:2 * F])
            nc.vector.tensor_reduce(abf[:], abm[:], AX, add)
            dot_ps = ps.tile([1, 1], fp32, tag="psC")
            nc.tensor.matmul(
                dot_ps[:], ones_fc[0:S, :], abf[:], start=True, stop=True
            )
            # out = [dot_k, sp_c, spp_c]; host assembles the loss
            out_sb = sb.tile([1, 3], fp32)
            nc.vector.tensor_copy(out_sb[:, 0:1], dot_ps[:])
            nc.vector.tensor_copy(out_sb[:, 1:3], sp_ps[:])
            nc.sync.dma_start(out_d.ap(), out_sb[:])

    nc.compile()
    return nc


def _get_nc():
    if "nc" not in _BUILT:
        _BUILT["nc"] = _build_bass()
    return _BUILT["nc"]


def _get_runner():
    """Cached jitted SPMD executor.

    Same lowering as concourse.bass2jax.run_bass_via_pjrt, but (a) the
    jitted callable is built once and reused, and (b) inputs are pre-placed
    with jax.device_put under an explicit sharding so per-core H2D staging
    overlaps instead of serializing inside the execute (the cross-core
    start skew is what the first collective's rendezvous waits on).
    """
    if "runner" in _BUILT:
        return _BUILT["runner"]

    import jax
    import concourse.mybir as mybir
    from concourse.bass2jax import (
        _bass_exec_p,
        install_neuronx_cc_hook,
        partition_id_tensor,
    )
    from jax.experimental.shard_map import shard_map
    from jax.sharding import Mesh, NamedSharding, PartitionSpec

    nc = _get_nc()
    install_neuronx_cc_hook()

    partition_name = (
        nc.partition_id_tensor.name if nc.partition_id_tensor else None
    )
    in_names, out_names, out_avals, zero_outs = [], [], [], []
    for alloc in nc.m.functions[0].allocations:
        if not isinstance(alloc, mybir.MemoryLocationSet):
            continue
        name = alloc.memorylocations[0].name
        if alloc.kind == "ExternalInput":
            if name != partition_name:
                in_names.append(name)
        elif alloc.kind == "ExternalOutput":
            shape = tuple(alloc.tensor_shape)
            dtype = mybir.dt.np(alloc.dtype)
            out_names.append(name)
            out_avals.append(jax.core.ShapedArray(shape, dtype))
            zero_outs.append(np.zeros(shape, dtype))
    n_params = len(in_names)
    all_names = in_names + out_names
    if partition_name is not None:
        all_names = all_names + [partition_name]
    donate = tuple(range(n_params, n_params + len(out_names)))

    def _body(*args):
        operands = list(args)
        if partition_name is not None:
            operands.append(partition_id_tensor())
        outs = _bass_exec_p.bind(
            *operands,
            out_avals=tuple(out_avals),
            in_names=tuple(all_names),
            out_names=tuple(out_names),
            lowering_input_output_aliases=(),
            sim_require_finite=True,
            sim_require_nnan=True,
            nc=nc,
        )
        return tuple(outs)

    devices = jax.devices()[:N_CORES]
    mesh = Mesh(np.asarray(devices), ("core",))
    spec = NamedSharding(mesh, PartitionSpec("core"))
    n_all = n_params + len(zero_outs)
    sharded = jax.jit(
        shard_map(
            _body, mesh=mesh,
            in_specs=(PartitionSpec("core"),) * n_all,
            out_specs=(PartitionSpec("core"),) * len(out_names),
            check_rep=False,
        ),
        donate_argnums=donate,
        keep_unused=True,
    )

    def run(in_maps):
        concat_in = [
            np.concatenate([np.asarray(m[name]) for m in in_maps], axis=0)
            for name in in_names
        ]
        concat_zero = [
            np.zeros((N_CORES * z.shape[0], *z.shape[1:]), z.dtype)
            for z in zero_outs
        ]
        args = [jax.device_put(a, spec) for a in concat_in + concat_zero]
        out_arrs = sharded(*args)
        return [
            {
                name: np.asarray(out_arrs[i]).reshape(
                    N_CORES, *out_avals[i].shape
                )[c]
                for i, name in enumerate(out_names)
            }
            for c in range(N_CORES)
        ]

    _BUILT["runner"] = run
    return run


def kernel(X1, X2):
    import ml_dtypes

    X1 = np.asarray(X1, dtype=np.float32).astype(ml_dtypes.bfloat16)
    X2 = np.asarray(X2, dtype=np.float32).astype(ml_dtypes.bfloat16)
    assert X1.shape == (N_TOTAL, F) and X2.shape == (N_TOTAL, F)

    run = _get_runner()
    in_maps = [
        {
            "x": np.concatenate(
                [X1[k * ROWS:(k + 1) * ROWS], X2[k * ROWS:(k + 1) * ROWS]]
            )
        }
        for k in range(N_CORES)
    ]
    results = run(in_maps)
    return combine(results)


def combine(results):
    """Sum the per-core partial scalars and finish the loss formula."""
    dot = sum(np.float64(r["out"][0, 0]) for r in results)
    sp = sum(np.float64(r["out"][0, 1]) for r in results)
    spp = sum(np.float64(r["out"][0, 2]) for r in results)
    n = float(N_TOTAL)
    return np.float32(1.0 - sp / n + (LAM / n) * (dot - spp))
